# revision 32
# baseline (speedup 1.0000x reference)
"""BoundaryMaxPooling Trainium2 kernel.

Reference computation (B=16, C2=512, T=Tf=126):
  - segment windows [s0,s1) / [e0,e1) derived from segments[0] only (batch-0 row)
  - out[b, c, t]      = max_{j in [s0(t), s1(t))} feature[b, c, j]       (c < 256)
  - out[b, 256+c, t]  = max_{j in [e0(t), e1(t))} feature[b, 256+c, j]

Device algorithm (per core, 2 batches, data-parallel over batch):
  Sparse-table (log-level) range max with j on SBUF partitions:
    L_0[j, c'] = feature^T   (c' = half*512 + b*256 + c, 1024 columns, bf16)
    L_{k+1}[j] = max(L_k[j], L_k[j + 2^k])   for j in [0, 127 - 2^{k+1})
  Partition shifts 1/2/4/8/16/32 are produced by the TensorEngine with
  exact one-hot band matrices (fp8e4 stationary x bf16 moving, fp32 PSUM);
  compute engines cannot read two SBUF operands at different base
  partitions, so every shift needs the PE. Window max for length L,
  k = floor(log2 L):
    out[t] = max(L_k[a(t)], L_k[b(t)]),  a = lo, b = hi - 2^k
  Both lookups are exact one-hot gather matmuls (fp8e4 one-hots) accumulated
  over levels in PSUM; a zero one-hot column contributes exact 0.  The final
  max of the two PSUM accumulators is a single DVE op per half writing the
  bf16 output tile.  Host precomputes all index matrices from segments[0]
  (replicated across cores), pre-transposes features per core (bf16), and
  reassembles/transposes the output; empty end-windows (e0 == -1) are
  data-independent and set to float32 min on the host, matching the
  reference.  All values stay exactly bf16 end-to-end on device (max never
  creates new values), so the only rounding is the host's fp32->bf16 cast.

Performance notes (measured ~26.8us HW exec; baseline of this session was
30.8us; ~10us is fixed NEFF preamble/teardown protocol and the input DMA
cannot land before ~9.5us):
  - Software-pipelined chain paced by the DVE: per level two 512-wide
    TensorTensor maxes (~650ns each, fp32 PSUM operand forces 1x mode)
    run back-to-back while the PE interleaves the next level's shift
    between gather fillers (1 shift + 2 gathers = 639ns per DVE slot).
    All 32 gather matmuls run inline; only the four level-6 ones trail.
  - Emission order matters twice: Tile is a STATIC scheduler that (a)
    honors priority (chain shifts/maxes high so an input-DMA-gated
    gather can never head-block the in-order PE queue) and (b) assigns
    count-based cross-engine semaphore waits at model-scheduling time,
    so consumers are emitted right after their producers and tail
    gathers stay at normal priority.
  - pk0 carries exactly the 2048B/row feature payload (one DMA packet
    per partition row) so its completion semaphore (gating chain start)
    fires earliest; shift band matrices are built on-device by gpsimd
    affine_select; levels 0-1 one-hot gathers ride in pk1 right behind.
  - fp8 one-hot x bf16 moving matmuls are exact (0/1 weights select bf16
    values into fp32 PSUM); per-(stream,half) PSUM accumulators avoid
    false WAR hazards between a half-1 gather and the half-0 staging.
  - fp32 warmup matmuls bridge the input wait: HAM un-throttles the PE
    (1.2 -> 2.4 GHz) only after ~3.4us of gap-free PE activity; 4
    matmuls (8 LOW/HIGH passes, ~427ns each) end right as pk0's
    semaphore fires.
  - Tail: DVE casts/maxes (TensorTensor reads only one PSUM operand, so
    acc0 stages through SBUF; the h1 staging runs on ACT, kept warm by
    two tiny paced reads — a cold ACT wakes ~0.5-0.9us late). Output
    halves go on the SP HWDGE ring (2D descriptor gen ~840ns vs ~1500ns
    on the ACT ring).
  - NOTE: the unused "shc" pool below is load-bearing: removing it
    shifts SBUF tile addresses and a latent overlap corrupts results.
    Do not remove without re-verifying numerics on hardware.
"""

import os
import sys

import numpy as np

if os.path.isdir("/opt/trn_rl_repo") and "/opt/trn_rl_repo" not in sys.path:
    sys.path.insert(0, "/opt/trn_rl_repo")

import concourse.bass as bass  # noqa: E402
from concourse import bacc, mybir, tile  # noqa: E402
from concourse.bass_utils import run_bass_kernel_spmd  # noqa: E402

B, C2, T = 16, 512, 126
C = C2 // 2  # 256
NCORES = 8
BPC = B // NCORES  # batches per core = 2
CPRIME = BPC * C2  # 1024 columns per core
NLEV = 7
KS = [127 - (1 << k) for k in range(NLEV)]  # valid rows of level k
NSHIFT = 6  # PE shift matmuls for levels 1..6 (compute engines cannot read
# SBUF at partition offsets other than 0 for tensor_tensor: both-SB inputs
# must share a base partition, so even the shift-by-32 needs the PE)
WCOL = 128  # every stationary matrix padded to 128 columns (enables FWL)
NWARM = 4  # fp32 warmup matmuls bridging engine-preamble -> input-landing

F32 = mybir.dt.float32
BF16 = mybir.dt.bfloat16
F8 = mybir.dt.float8e4
U8 = mybir.dt.uint8
MAX = mybir.AluOpType.max

_CACHE = {}

# test.py hooks: set TRACE=True before calling kernel() to capture a profile.
TRACE = False
LAST_RESULTS = None


FT_BYTES = CPRIME * 2  # 2048 B of bf16 feature row per partition


def _w8_layout():
    """Byte offsets of each fp8 matrix inside the packed uint8 tensors.

    DMA throughput here is packet-bound (~175ns per partition-row packet
    regardless of size), so inputs are packed into as few fat-row tensors
    as possible and bitcast on device:
      pk0 [T, FT_BYTES + 6*WCOL]: ft (bf16 bytes) + the 6 shift matrices —
          everything the level chain needs, lands first.
      pk1 [T, 16*WCOL]: gather one-hots, levels 0-3.
      pk2 [T, 12*WCOL]: gather one-hots, levels 4-6.
    All three go on the same HWDGE ring in order, so pk0 gets the full
    16-engine bandwidth and the gathers stream in during the chain.
    Returns ({key: (tensor_idx, byte_off)}, [nbytes0, nbytes1, nbytes2]).
    """
    offs = {}
    nbytes = [FT_BYTES, 0, 0, 0]
    # pk0 carries ONLY the feature bytes (2048 B/row = one DMA packet per
    # partition row) so its completion semaphore — which gates the chain
    # start — fires earliest; the shift band matrices are generated
    # on-device (affine_select) and never DMAed. Levels 0-1 one-hots
    # ride in pk1 right behind it: their gathers run in the first chain
    # slots and later tensors' semaphores fire too late (~1us+ after
    # last byte) to gate them without stalling the in-order PE queue.
    for k in range(NLEV):
        ti = 1 if k < 2 else (2 if k < 4 else 3)
        for gi in range(2):
            for h in range(2):
                offs[("g", gi, h, k)] = (ti, nbytes[ti])
                nbytes[ti] += WCOL
    return offs, nbytes


def _build_module():
    nc = bacc.Bacc(None, target_bir_lowering=False, debug=False)

    offs, nbytes = _w8_layout()
    pk_ins = [
        nc.dram_tensor(f"pk{i}", [T, nbytes[i]], U8, kind="ExternalInput")
        for i in range(4)
    ]
    # Output padded to 128 rows: full-partition DMAs keep the HWDGE
    # descriptor path uniform (rows 126-127 are junk; host ignores them).
    out = nc.dram_tensor("out", [128, CPRIME], BF16, kind="ExternalOutput")

    with tile.TileContext(nc) as tc:
        with (
            tc.tile_pool(name="lv", bufs=1) as lvp,
            tc.tile_pool(name="gw", bufs=1) as gwp,
            tc.tile_pool(name="acc", bufs=1, space=bass.MemorySpace.PSUM) as accp,
            tc.tile_pool(name="shp", bufs=4, space=bass.MemorySpace.PSUM) as shpp,
            tc.tile_pool(name="shc", bufs=2) as shcp,
        ):
            pk = [
                gwp.tile([T, nbytes[i]], U8, name=f"pk{i}") for i in range(4)
            ]
            # All three on the sync ring, in order: pk0 (ft + shifts) gets
            # the full 16-SDMA-engine bandwidth first; the gather one-hots
            # stream in behind it while the chain runs.
            for i in range(4):
                nc.sync.dma_start(out=pk[i][:, :], in_=pk_ins[i][:, :])

            ft = pk[0][:, 0:FT_BYTES].bitcast(BF16)

            # The shift band matrices (one-hot at [c + 2^k, c]) are pure
            # index functions — build them on-device on the otherwise-idle
            # gpsimd engine instead of shipping them through the input DMA
            # (keeps pk0 at exactly one packet per partition row).
            ones = gwp.tile([128, WCOL], BF16, name="ones")
            nc.gpsimd.memset(ones[:, :], 1.0)
            shm = []
            for k in range(NSHIFT):
                m = gwp.tile([128, WCOL], BF16, name=f"shm{k}")
                # keep 1.0 where (j - c - 2^k) == 0, else 0.0
                nc.gpsimd.affine_select(
                    out=m[:, :],
                    in_=ones[:, :],
                    compare_op=mybir.AluOpType.is_equal,
                    fill=0.0,
                    base=-(1 << k),
                    pattern=[[-1, WCOL]],
                    channel_multiplier=1,
                )
                shm.append(m)

            def sh_ap(k):
                return shm[k][0 : KS[k], 0:WCOL]

            def g_ap(gi, h, k):
                ti, o = offs[("g", gi, h, k)]
                return pk[ti][0 : KS[k], o : o + WCOL].bitcast(F8)

            L = [None] + [
                lvp.tile([KS[k], CPRIME], BF16, name=f"L{k}")[:, :]
                for k in range(1, NLEV)
            ]

            def L_ap(k, h, rows):
                if k == 0:
                    return ft[0:rows, h * 512 : (h + 1) * 512]
                return L[k][0:rows, h * 512 : (h + 1) * 512]

            # Separate PSUM tiles per (stream, half): a single [128, 1024]
            # tile per stream creates false WAR hazards (Tile serializes a
            # half-1 gather against a half-0 staging read of the same tile).
            p_acc = [
                [
                    accp.tile([128, 512], F32, name=f"pacc{gi}_{h}")
                    for h in range(2)
                ]
                for gi in range(2)
            ]

            # PE warmup: HAM throttles the PE to half clock until it has
            # been continuously busy ~3.4-4.6us (run-to-run variance). The
            # warmup reads the framework's const-zero tensor (memset during
            # Bass init, BEFORE the ~7.2us post-init all-engine barrier),
            # so LDWEIGHTS can start the moment the Tensor sequencer's
            # preamble drains (~6.7us) with no body-phase memset gating it.
            # fp32 matmuls (LOW+HIGH double pass, ~854ns each at half
            # clock) keep the busy window gap-free.
            wzs = nc.const_aps.tensor(0.0, (128, 128), F32)
            wzm = nc.const_aps.tensor(0.0, (128, 256), F32)
            for _ in range(NWARM - 1):
                nc.tensor.matmul(
                    p_acc[0][0][0:128, 0:256],
                    wzs,
                    wzm,
                    start=True,
                    stop=True,
                )
            # Final warmup at half width: 3x854 + 428 ~= the 3.4us HAM
            # window, so the chain starts the moment the PE is unthrottled
            # instead of ~0.4us later.
            nc.tensor.matmul(
                p_acc[0][0][0:128, 0:128],
                wzs,
                wzm[:, 0:128],
                start=True,
                stop=True,
            )

            # Software-pipelined chain. The DVE paces the kernel: per level
            # two 512-wide TensorTensor maxes (~640ns each, fp32 PSUM
            # operand forces 1x mode). Emission order = engine queue order,
            # so the PE queue interleaves each next shift between gather
            # fillers such that the PE never head-blocks the chain:
            #   PE:  sh_k_h0 | g g | sh_k_h1 | g g | sh_{k+1}_h0 ...
            #   DVE:     max_{k-1}_h1 | max_k_h0  | max_k_h1 ...
            # Each DVE slot (~640ns) covers one shift + two gathers
            # (3 x 213ns) on the PE. All 28 gather matmuls run inline.
            # Shifts and chain maxes are high_priority: Tile's static
            # scheduler must never place a (possibly input-DMA-gated)
            # gather ahead of the chain on the in-order PE queue.
            def emit_shift(k, h):
                shp = shpp.tile([128, 512], F32, name=f"shp{k}_{h}", tag="shp")
                with tc.high_priority():
                    nc.tensor.matmul(
                        shp[:, :],
                        sh_ap(k),
                        L_ap(k, h, KS[k]),
                        start=True,
                        stop=True,
                    )
                return shp

            def emit_max(k, h, shp):
                sl = slice(h * 512, (h + 1) * 512)
                with tc.high_priority():
                    nc.vector.tensor_max(
                        L[k + 1][:, sl],
                        L_ap(k, h, KS[k + 1]),
                        shp[0 : KS[k + 1], :],
                    )

            def emit_gather(k, gi, h):
                nc.tensor.matmul(
                    p_acc[gi][h][:, :],
                    g_ap(gi, h, k),
                    L_ap(k, h, KS[k]),
                    start=(k == 0),
                    stop=(k == NLEV - 1),
                )

            # Gather filler schedule: at the level-k window run gathers one
            # level behind (k-1 gi0 / gi1), so a late pk1 landing can never
            # head-block the early shifts.
            sh00 = emit_shift(0, 0)
            sh01 = emit_shift(0, 1)
            emit_max(0, 0, sh00)
            shp_h0 = emit_shift(1, 0)
            emit_gather(0, 0, 0)
            emit_gather(0, 0, 1)
            emit_max(0, 1, sh01)
            for k in range(1, NSHIFT):
                # PE: sh_k_h1 then two fillers; DVE: max_k_h0.
                shp_h1 = emit_shift(k, 1)
                emit_max(k, 0, shp_h0)
                emit_gather(k - 1, 1, 0)
                emit_gather(k - 1, 1, 1)
                # PE: sh_{k+1}_h0 then two fillers; DVE: max_k_h1.
                if k + 1 < NSHIFT:
                    shp_h0 = emit_shift(k + 1, 0)
                emit_max(k, 1, shp_h1)
                emit_gather(k, 0, 0)
                emit_gather(k, 0, 1)
                if k == NSHIFT - 1:
                    # Last level: no next shift to pace against — inline the
                    # level-5 gi1 gathers too. Tail matmuls run ~320ns (the
                    # sem-gated dispatch breaks weight-load overlap), so
                    # every gather moved out of the tail shortens it.
                    emit_gather(k, 1, 0)
                    emit_gather(k, 1, 1)

            # Tail. The h0-feeding level-6 gathers only need max5_h0, so
            # they run on the PE while the DVE still does max5_h1; the ACT
            # copies are emitted immediately after the gather that closes
            # their accumulator half (Tile assigns the PE-count wait at
            # emission point, so emitting late = starting late).
            # TensorTensor may read only one PSUM operand, hence acc0 is
            # staged to SBUF on ACT (fp32 PSUM source: ~686ns per half).
            s1t = gwp.tile([T, CPRIME], BF16, name="s1t")
            ot = gwp.tile([128, CPRIME], BF16, name="ot")
            # rows 126-127 are never computed but are DMAed (padding);
            # Tile requires them written before read, and gpsimd memsets
            # must start 16-partition-aligned — clear the whole tile early
            # (the final maxes overwrite rows 0:126).
            nc.gpsimd.memset(ot[:, :], 0.0)
            sl0 = slice(0, 512)
            sl1 = slice(512, 1024)
            # The whole tail runs on the DVE: the ACT engine wakes ~0.9us
            # after its semaphore fires (vs ~40ns for the DVE), so ACT
            # staging copies lose more to latency than a second engine
            # buys in overlap. PE order closes each accumulator half at
            # its earliest data-ready point (h0 halves need only max5_h0
            # and run while the DVE still computes max5_h1).
            # Tail gathers stay at NORMAL priority: Tile assigns count-based
            # semaphore waits at model-scheduling time, and high-priority
            # gathers get scheduled ahead of the DVE tail ops, inflating
            # their PE-count waits ~0.7us past the true producer.
            emit_gather(NSHIFT, 0, 0)      # g6 gi0 h0 -> acc0_h0 closed
            emit_gather(NSHIFT, 1, 0)      # g6 gi1 h0 -> acc1_h0 closed
            emit_gather(NSHIFT, 0, 1)      # g6 gi0 h1 -> acc0_h1 closed
            emit_gather(NSHIFT, 1, 1)      # g6 gi1 h1 -> acc1_h1 closed
            # ACT stages the h1 half in parallel with the DVE's h0 work.
            # The ACT engine wakes ~0.5-0.9us after a semaphore when it has
            # been idle, so two tiny paced reads keep it warm through the
            # chain's last levels.
            warm = gwp.tile([1, 8], BF16, name="warm")
            nc.scalar.copy(out=warm[:, :], in_=L[NSHIFT - 1][0:1, 0:8])
            nc.scalar.copy(out=warm[:, :], in_=L[NSHIFT][0:1, 0:8])
            nc.scalar.copy(out=s1t[:, sl1], in_=p_acc[0][1][0:T, :])
            with tc.high_priority():
                nc.vector.tensor_copy(s1t[:, sl0], p_acc[0][0][0:T, :])
                nc.vector.tensor_max(ot[0:T, sl0], s1t[:, sl0], p_acc[1][0][0:T, :])
                # Both halves on the SP HWDGE ring: its sequencer generates
                # the 2D descriptors in ~840ns vs ~1500ns on the ACT ring.
                nc.sync.dma_start(out=out[:, sl0], in_=ot[:, sl0])
                nc.vector.tensor_max(ot[0:T, sl1], s1t[:, sl1], p_acc[1][1][0:T, :])
                nc.sync.dma_start(out=out[:, sl1], in_=ot[:, sl1])

    nc.compile()
    return nc


def _host_windows(segments):
    """Replicates the reference's index math on segments[0]. Returns per half
    (lo, hi) clamped windows plus the empty mask."""
    seg = np.clip(segments.astype(np.float32), 0.0, 125.0)
    row = seg[0]  # [T, 4]
    s0 = np.floor(row[:, 0]).astype(np.int32)
    s1 = np.ceil(row[:, 1]).astype(np.int32)
    s1 = np.where(s0 == s1, s1 + 1, s1)
    e0 = np.floor(row[:, 2]).astype(np.int32)
    e1 = np.ceil(row[:, 3]).astype(np.int32)
    e0 = np.where(e0 == e1, e0 - 1, e0)

    halves = []
    for lo, hi in ((s0, s1), (e0, e1)):
        lo_c = np.maximum(lo, 0)
        hi_c = np.minimum(hi, T)
        empty = lo_c >= hi_c
        halves.append((lo_c, hi_c, empty))
    return halves


def _host_pk(segments):
    """Packed uint8 input tensors (fp8 one-hot bytes; pk0 also carries ft
    bytes which the caller fills per core)."""
    halves = _host_windows(segments)
    offs, nbytes = _w8_layout()
    one = mybir.dt.np(F8)(1.0).view(np.uint8)  # fp8e4 bit pattern of 1.0
    pk = [np.zeros((T, n), np.uint8) for n in nbytes]
    for h, (lo, hi, empty) in enumerate(halves):
        for t in range(T):
            if empty[t]:
                continue
            ln = int(hi[t] - lo[t])
            k = ln.bit_length() - 1
            a = int(lo[t])
            b = int(hi[t]) - (1 << k)
            ta, oa = offs[("g", 0, h, k)]
            tb, ob = offs[("g", 1, h, k)]
            pk[ta][a, oa + t] = one
            pk[tb][b, ob + t] = one
    return pk, halves


def _shard_feature(feature):
    """Core i gets batches [2i, 2i+2) as bf16 [T, CPRIME] with
    c' = half*512 + local_batch*256 + channel_within_half."""
    bf = mybir.dt.np(BF16)
    fts = []
    for i in range(NCORES):
        pair = feature[BPC * i : BPC * (i + 1)]
        arr = pair.reshape(BPC, 2, C, T)  # [b, h, c, j]
        arr = np.ascontiguousarray(arr.transpose(3, 1, 0, 2).reshape(T, CPRIME))
        fts.append(arr.astype(bf))
    return fts


def _unshard(results, halves):
    out = np.empty((B, C2, T), np.float32)
    for i in range(NCORES):
        r = np.asarray(results[i]["out"])[0:T].astype(np.float32)  # [T, CPRIME]
        arr = r.reshape(T, 2, BPC, C).transpose(2, 1, 3, 0)  # [b, h, c, t]
        out[BPC * i : BPC * (i + 1)] = arr.reshape(BPC, C2, T)
    neg = np.finfo(np.float32).min
    for h, (_, _, empty) in enumerate(halves):
        if empty.any():
            out[:, h * C : (h + 1) * C, empty] = neg
    return out


def kernel(feature, segments):
    global LAST_RESULTS
    feature = np.ascontiguousarray(feature, dtype=np.float32)
    segments = np.ascontiguousarray(segments, dtype=np.float32)

    if "nc" not in _CACHE:
        _CACHE["nc"] = _build_module()
    nc = _CACHE["nc"]

    pk, halves = _host_pk(segments)
    fts = _shard_feature(feature)

    in_maps = []
    for i in range(NCORES):
        pk0 = pk[0].copy()
        pk0[:, 0:FT_BYTES] = fts[i].view(np.uint8)
        in_maps.append(
            {"pk0": pk0, "pk1": pk[1], "pk2": pk[2], "pk3": pk[3]}
        )

    res = run_bass_kernel_spmd(nc, in_maps, list(range(NCORES)), trace=TRACE)
    LAST_RESULTS = res
    return _unshard(res.results, halves)



# revision 36
# speedup vs baseline: 1.0214x; 1.0214x over previous
"""BoundaryMaxPooling Trainium2 kernel.

Reference computation (B=16, C2=512, T=Tf=126):
  - segment windows [s0,s1) / [e0,e1) derived from segments[0] only (batch-0 row)
  - out[b, c, t]      = max_{j in [s0(t), s1(t))} feature[b, c, j]       (c < 256)
  - out[b, 256+c, t]  = max_{j in [e0(t), e1(t))} feature[b, 256+c, j]

Device algorithm (per core, 2 batches, data-parallel over batch):
  Sparse-table (log-level) range max with j on SBUF partitions:
    L_0[j, c'] = feature^T   (c' = half*512 + b*256 + c, 1024 columns, bf16)
    L_{k+1}[j] = max(L_k[j], L_k[j + 2^k])   for j in [0, 127 - 2^{k+1})
  Partition shifts 1/2/4/8/16/32 are produced by the TensorEngine with
  exact one-hot band matrices (bf16 stationary built on-device by gpsimd
  affine_select x bf16 moving, fp32 PSUM); compute engines cannot read
  two SBUF operands at different base partitions, so every shift needs
  the PE. Window max for length L, k = floor(log2 L):
    out[t] = max(L_k[a(t)], L_k[b(t)]),  a = lo, b = hi - 2^k
  Both lookups are exact one-hot gather matmuls (fp8e4 one-hots) accumulated
  over levels in PSUM; a zero one-hot column contributes exact 0.  The final
  max of the two PSUM accumulators is a single DVE op per half writing the
  bf16 output tile.  Host precomputes all index matrices from segments[0]
  (replicated across cores), pre-transposes features per core (bf16), and
  reassembles/transposes the output; empty end-windows (e0 == -1) are
  data-independent and set to float32 min on the host, matching the
  reference.  All values stay exactly bf16 end-to-end on device (max never
  creates new values), so the only rounding is the host's fp32->bf16 cast.

Performance notes (measured ~26.8us HW exec; baseline of this session was
30.8us; ~10us is fixed NEFF preamble/teardown protocol and the input DMA
cannot land before ~9.5us):
  - Software-pipelined chain paced by the DVE: per level two 512-wide
    TensorTensor maxes (~650ns each, fp32 PSUM operand forces 1x mode)
    run back-to-back while the PE interleaves the next level's shift
    between gather fillers (1 shift + 2 gathers = 639ns per DVE slot).
    All 32 gather matmuls run inline; only the four level-6 ones trail.
  - Emission order matters twice: Tile is a STATIC scheduler that (a)
    honors priority (chain shifts/maxes high so an input-DMA-gated
    gather can never head-block the in-order PE queue) and (b) assigns
    count-based cross-engine semaphore waits at model-scheduling time,
    so consumers are emitted right after their producers and tail
    gathers stay at normal priority.
  - pk0 carries exactly the 2048B/row feature payload (one DMA packet
    per partition row) so its completion semaphore (gating chain start)
    fires earliest; shift band matrices are built on-device by gpsimd
    affine_select; levels 0-1 one-hot gathers ride in pk1 right behind.
  - fp8 one-hot x bf16 moving matmuls are exact (0/1 weights select bf16
    values into fp32 PSUM); per-(stream,half) PSUM accumulators avoid
    false WAR hazards between a half-1 gather and the half-0 staging.
  - fp32 warmup matmuls bridge the input wait: HAM un-throttles the PE
    (1.2 -> 2.4 GHz) only after ~3.4us of gap-free PE activity; 3 full
    + 1 half-width matmul (~3.4us of LOW/HIGH passes) end right as
    pk0's semaphore fires.
  - Tail: DVE casts/maxes (TensorTensor reads only one PSUM operand, so
    acc0 stages through SBUF; the h1 staging runs on ACT, kept warm by
    two tiny paced reads — a cold ACT wakes ~0.5-0.9us late). Output
    halves go on the SP HWDGE ring (2D descriptor gen ~840ns vs ~1500ns
    on the ACT ring).
  - NOTE: the unused "shc" pool below is load-bearing: removing it
    shifts SBUF tile addresses and a latent overlap corrupts results.
    Do not remove without re-verifying numerics on hardware.
"""

import os
import sys

import numpy as np

if os.path.isdir("/opt/trn_rl_repo") and "/opt/trn_rl_repo" not in sys.path:
    sys.path.insert(0, "/opt/trn_rl_repo")

import concourse.bass as bass  # noqa: E402
from concourse import bacc, mybir, tile  # noqa: E402
from concourse.bass_utils import run_bass_kernel_spmd  # noqa: E402

B, C2, T = 16, 512, 126
C = C2 // 2  # 256
NCORES = 8
BPC = B // NCORES  # batches per core = 2
CPRIME = BPC * C2  # 1024 columns per core
NLEV = 7
KS = [127 - (1 << k) for k in range(NLEV)]  # valid rows of level k
NSHIFT = 6  # PE shift matmuls for levels 1..6 (compute engines cannot read
# SBUF at partition offsets other than 0 for tensor_tensor: both-SB inputs
# must share a base partition, so even the shift-by-32 needs the PE)
WCOL = 128  # every stationary matrix padded to 128 columns (enables FWL)
NWARM = 4  # fp32 warmup matmuls bridging engine-preamble -> input-landing

F32 = mybir.dt.float32
BF16 = mybir.dt.bfloat16
F8 = mybir.dt.float8e4
U8 = mybir.dt.uint8
MAX = mybir.AluOpType.max

_CACHE = {}

# test.py hooks: set TRACE=True before calling kernel() to capture a profile.
TRACE = False
LAST_RESULTS = None


FT_BYTES = CPRIME * 2  # 2048 B of bf16 feature row per partition


def _w8_layout():
    """Byte offsets of each fp8 one-hot matrix inside the packed uint8
    input tensors (DMA cost is per-partition-row packet-bound):
      pk0 [T, FT_BYTES]: the bf16 feature rows, exactly one packet/row.
      pk1 [T,  8*WCOL]: gather one-hots, levels 0-1 (needed earliest).
      pk2 [T,  8*WCOL]: gather one-hots, levels 2-3.
      pk3 [T, 12*WCOL]: gather one-hots, levels 4-6.
    All four go on the same HWDGE ring in order, so pk0 gets the full
    16-engine bandwidth and the gathers stream in during the chain.
    Returns ({key: (tensor_idx, byte_off)}, [nbytes...]).
    """
    offs = {}
    nbytes = [FT_BYTES, 0, 0, 0]
    # pk0 carries ONLY the feature bytes (2048 B/row = one DMA packet per
    # partition row) so its completion semaphore — which gates the chain
    # start — fires earliest; the shift band matrices are generated
    # on-device (affine_select) and never DMAed. Levels 0-1 one-hots
    # ride in pk1 right behind it: their gathers run in the first chain
    # slots and later tensors' semaphores fire too late (~1us+ after
    # last byte) to gate them without stalling the in-order PE queue.
    for k in range(NLEV):
        ti = 1 if k < 2 else (2 if k < 4 else 3)
        for gi in range(2):
            for h in range(2):
                offs[("g", gi, h, k)] = (ti, nbytes[ti])
                nbytes[ti] += WCOL
    return offs, nbytes


def _build_module():
    nc = bacc.Bacc(None, target_bir_lowering=False, debug=False)

    offs, nbytes = _w8_layout()
    pk_ins = [
        nc.dram_tensor(f"pk{i}", [T, nbytes[i]], U8, kind="ExternalInput")
        for i in range(4)
    ]
    # Output padded to 128 rows: full-partition DMAs keep the HWDGE
    # descriptor path uniform (rows 126-127 are junk; host ignores them).
    out = nc.dram_tensor("out", [128, CPRIME], BF16, kind="ExternalOutput")

    with tile.TileContext(nc) as tc:
        with (
            tc.tile_pool(name="lv", bufs=1) as lvp,
            tc.tile_pool(name="gw", bufs=1) as gwp,
            tc.tile_pool(name="acc", bufs=1, space=bass.MemorySpace.PSUM) as accp,
            tc.tile_pool(name="shp", bufs=4, space=bass.MemorySpace.PSUM) as shpp,
            tc.tile_pool(name="shc", bufs=2) as shcp,
        ):
            pk = [
                gwp.tile([T, nbytes[i]], U8, name=f"pk{i}") for i in range(4)
            ]
            # All four on the sync ring, in order: pk0 (feature rows) gets
            # the full 16-SDMA-engine bandwidth first; the gather one-hots
            # stream in behind it while the chain runs.
            for i in range(4):
                nc.sync.dma_start(out=pk[i][:, :], in_=pk_ins[i][:, :])

            ft = pk[0][:, 0:FT_BYTES].bitcast(BF16)

            # The shift band matrices (one-hot at [c + 2^k, c]) are pure
            # index functions — build them on-device on the otherwise-idle
            # gpsimd engine instead of shipping them through the input DMA
            # (keeps pk0 at exactly one packet per partition row).
            ones = gwp.tile([128, WCOL], BF16, name="ones")
            nc.gpsimd.memset(ones[:, :], 1.0)
            shm = []
            for k in range(NSHIFT):
                m = gwp.tile([128, WCOL], BF16, name=f"shm{k}")
                # keep 1.0 where (j - c - 2^k) == 0, else 0.0
                nc.gpsimd.affine_select(
                    out=m[:, :],
                    in_=ones[:, :],
                    compare_op=mybir.AluOpType.is_equal,
                    fill=0.0,
                    base=-(1 << k),
                    pattern=[[-1, WCOL]],
                    channel_multiplier=1,
                )
                shm.append(m)

            def sh_ap(k):
                return shm[k][0 : KS[k], 0:WCOL]

            def g_ap(gi, h, k):
                ti, o = offs[("g", gi, h, k)]
                return pk[ti][0 : KS[k], o : o + WCOL].bitcast(F8)

            L = [None] + [
                lvp.tile([KS[k], CPRIME], BF16, name=f"L{k}")[:, :]
                for k in range(1, NLEV)
            ]

            def L_ap(k, h, rows):
                if k == 0:
                    return ft[0:rows, h * 512 : (h + 1) * 512]
                return L[k][0:rows, h * 512 : (h + 1) * 512]

            # Separate PSUM tiles per (stream, half): a single [128, 1024]
            # tile per stream creates false WAR hazards (Tile serializes a
            # half-1 gather against a half-0 staging read of the same tile).
            p_acc = [
                [
                    accp.tile([128, 512], F32, name=f"pacc{gi}_{h}")
                    for h in range(2)
                ]
                for gi in range(2)
            ]

            # PE warmup: HAM throttles the PE to half clock until it has
            # been continuously busy ~3.4-4.6us (run-to-run variance). The
            # warmup reads the framework's const-zero tensor (memset during
            # Bass init, BEFORE the ~7.2us post-init all-engine barrier),
            # so LDWEIGHTS can start the moment the Tensor sequencer's
            # preamble drains (~6.7us) with no body-phase memset gating it.
            # fp32 matmuls (LOW+HIGH double pass, ~854ns each at half
            # clock) keep the busy window gap-free.
            wzs = nc.const_aps.tensor(0.0, (128, 128), F32)
            wzm = nc.const_aps.tensor(0.0, (128, 256), F32)
            for _ in range(NWARM - 1):
                nc.tensor.matmul(
                    p_acc[0][0][0:128, 0:256],
                    wzs,
                    wzm,
                    start=True,
                    stop=True,
                )
            # Final warmup at half width: 3x854 + 428 ~= the 3.4us HAM
            # window, so the chain starts the moment the PE is unthrottled
            # instead of ~0.4us later.
            nc.tensor.matmul(
                p_acc[0][0][0:128, 0:128],
                wzs,
                wzm[:, 0:128],
                start=True,
                stop=True,
            )

            # Software-pipelined chain. The DVE paces the kernel: per level
            # two 512-wide TensorTensor maxes (~640ns each, fp32 PSUM
            # operand forces 1x mode). Emission order = engine queue order,
            # so the PE queue interleaves each next shift between gather
            # fillers such that the PE never head-blocks the chain:
            #   PE:  sh_k_h0 | g g | sh_k_h1 | g g | sh_{k+1}_h0 ...
            #   DVE:     max_{k-1}_h1 | max_k_h0  | max_k_h1 ...
            # Each DVE slot (~640ns) covers one shift + two gathers
            # (3 x 213ns) on the PE. All 28 gather matmuls run inline.
            # Shifts and chain maxes are high_priority: Tile's static
            # scheduler must never place a (possibly input-DMA-gated)
            # gather ahead of the chain on the in-order PE queue.
            def emit_shift(k, h):
                shp = shpp.tile([128, 512], F32, name=f"shp{k}_{h}", tag="shp")
                with tc.high_priority():
                    nc.tensor.matmul(
                        shp[:, :],
                        sh_ap(k),
                        L_ap(k, h, KS[k]),
                        start=True,
                        stop=True,
                    )
                return shp

            def emit_max(k, h, shp):
                sl = slice(h * 512, (h + 1) * 512)
                with tc.high_priority():
                    nc.vector.tensor_max(
                        L[k + 1][:, sl],
                        L_ap(k, h, KS[k + 1]),
                        shp[0 : KS[k + 1], :],
                    )

            def emit_gather(k, gi, h):
                nc.tensor.matmul(
                    p_acc[gi][h][:, :],
                    g_ap(gi, h, k),
                    L_ap(k, h, KS[k]),
                    start=(k == 0),
                    stop=(k == NLEV - 1),
                )

            # Gather filler schedule: at the level-k window run gathers one
            # level behind (k-1 gi0 / gi1), so a late pk1 landing can never
            # head-block the early shifts.
            sh00 = emit_shift(0, 0)
            sh01 = emit_shift(0, 1)
            emit_max(0, 0, sh00)
            shp_h0 = emit_shift(1, 0)
            emit_gather(0, 0, 0)
            emit_gather(0, 0, 1)
            emit_max(0, 1, sh01)
            for k in range(1, NSHIFT):
                # PE: sh_k_h1 then two fillers; DVE: max_k_h0.
                shp_h1 = emit_shift(k, 1)
                emit_max(k, 0, shp_h0)
                emit_gather(k - 1, 1, 0)
                emit_gather(k - 1, 1, 1)
                # PE: sh_{k+1}_h0 then two fillers; DVE: max_k_h1.
                if k + 1 < NSHIFT:
                    shp_h0 = emit_shift(k + 1, 0)
                emit_max(k, 1, shp_h1)
                emit_gather(k, 0, 0)
                emit_gather(k, 0, 1)
                if k == NSHIFT - 1:
                    # Last level: no next shift to pace against — inline the
                    # level-5 gi1 gathers too. Tail matmuls run ~320ns (the
                    # sem-gated dispatch breaks weight-load overlap), so
                    # every gather moved out of the tail shortens it.
                    emit_gather(k, 1, 0)
                    emit_gather(k, 1, 1)

            # Tail. The h0-feeding level-6 gathers only need max5_h0, so
            # they run on the PE while the DVE still does max5_h1; the ACT
            # copies are emitted immediately after the gather that closes
            # their accumulator half (Tile assigns the PE-count wait at
            # emission point, so emitting late = starting late).
            # TensorTensor may read only one PSUM operand, hence acc0 is
            # staged to SBUF on ACT (fp32 PSUM source: ~686ns per half).
            s1t = gwp.tile([T, CPRIME], BF16, name="s1t")
            ot = gwp.tile([128, CPRIME], BF16, name="ot")
            # rows 126-127 are never computed but are DMAed (padding);
            # Tile requires them written before read, and gpsimd memsets
            # must start 16-partition-aligned — clear the whole tile early
            # (the final maxes overwrite rows 0:126).
            nc.gpsimd.memset(ot[:, :], 0.0)
            sl0 = slice(0, 512)
            sl1 = slice(512, 1024)
            # The whole tail runs on the DVE: the ACT engine wakes ~0.9us
            # after its semaphore fires (vs ~40ns for the DVE), so ACT
            # staging copies lose more to latency than a second engine
            # buys in overlap. PE order closes each accumulator half at
            # its earliest data-ready point (h0 halves need only max5_h0
            # and run while the DVE still computes max5_h1).
            # Tail gathers stay at NORMAL priority: Tile assigns count-based
            # semaphore waits at model-scheduling time, and high-priority
            # gathers get scheduled ahead of the DVE tail ops, inflating
            # their PE-count waits ~0.7us past the true producer.
            emit_gather(NSHIFT, 0, 0)      # g6 gi0 h0 -> acc0_h0 closed
            emit_gather(NSHIFT, 1, 0)      # g6 gi1 h0 -> acc1_h0 closed
            emit_gather(NSHIFT, 0, 1)      # g6 gi0 h1 -> acc0_h1 closed
            emit_gather(NSHIFT, 1, 1)      # g6 gi1 h1 -> acc1_h1 closed
            # ACT stages the h1 half in parallel with the DVE's h0 work.
            # The ACT engine wakes ~0.5-0.9us after a semaphore when it has
            # been idle, so two tiny paced reads keep it warm through the
            # chain's last levels.
            warm = gwp.tile([1, 8], BF16, name="warm")
            nc.scalar.copy(out=warm[:, :], in_=L[NSHIFT - 1][0:1, 0:8])
            nc.scalar.copy(out=warm[:, :], in_=L[NSHIFT][0:1, 0:8])
            nc.scalar.copy(out=s1t[:, sl1], in_=p_acc[0][1][0:T, :])
            with tc.high_priority():
                nc.vector.tensor_copy(s1t[:, sl0], p_acc[0][0][0:T, :])
                nc.vector.tensor_max(ot[0:T, sl0], s1t[:, sl0], p_acc[1][0][0:T, :])
                # Both halves on the SP HWDGE ring: its sequencer generates
                # the 2D descriptors in ~840ns vs ~1500ns on the ACT ring.
                nc.sync.dma_start(out=out[:, sl0], in_=ot[:, sl0])
                nc.vector.tensor_max(ot[0:T, sl1], s1t[:, sl1], p_acc[1][1][0:T, :])
                nc.sync.dma_start(out=out[:, sl1], in_=ot[:, sl1])

    nc.compile()
    return nc


def _host_windows(segments):
    """Replicates the reference's index math on segments[0]. Returns per half
    (lo, hi) clamped windows plus the empty mask."""
    seg = np.clip(segments.astype(np.float32), 0.0, 125.0)
    row = seg[0]  # [T, 4]
    s0 = np.floor(row[:, 0]).astype(np.int32)
    s1 = np.ceil(row[:, 1]).astype(np.int32)
    s1 = np.where(s0 == s1, s1 + 1, s1)
    e0 = np.floor(row[:, 2]).astype(np.int32)
    e1 = np.ceil(row[:, 3]).astype(np.int32)
    e0 = np.where(e0 == e1, e0 - 1, e0)

    halves = []
    for lo, hi in ((s0, s1), (e0, e1)):
        lo_c = np.maximum(lo, 0)
        hi_c = np.minimum(hi, T)
        empty = lo_c >= hi_c
        halves.append((lo_c, hi_c, empty))
    return halves


def _host_pk(segments):
    """Packed uint8 input tensors (fp8 one-hot bytes; pk0 also carries ft
    bytes which the caller fills per core)."""
    halves = _host_windows(segments)
    offs, nbytes = _w8_layout()
    one = mybir.dt.np(F8)(1.0).view(np.uint8)  # fp8e4 bit pattern of 1.0
    pk = [np.zeros((T, n), np.uint8) for n in nbytes]
    for h, (lo, hi, empty) in enumerate(halves):
        for t in range(T):
            if empty[t]:
                continue
            ln = int(hi[t] - lo[t])
            k = ln.bit_length() - 1
            a = int(lo[t])
            b = int(hi[t]) - (1 << k)
            ta, oa = offs[("g", 0, h, k)]
            tb, ob = offs[("g", 1, h, k)]
            pk[ta][a, oa + t] = one
            pk[tb][b, ob + t] = one
    return pk, halves


def _shard_feature(feature):
    """Core i gets batches [2i, 2i+2) as bf16 [T, CPRIME] with
    c' = half*512 + local_batch*256 + channel_within_half."""
    bf = mybir.dt.np(BF16)
    fts = []
    for i in range(NCORES):
        pair = feature[BPC * i : BPC * (i + 1)]
        arr = pair.reshape(BPC, 2, C, T)  # [b, h, c, j]
        arr = np.ascontiguousarray(arr.transpose(3, 1, 0, 2).reshape(T, CPRIME))
        fts.append(arr.astype(bf))
    return fts


def _unshard(results, halves):
    out = np.empty((B, C2, T), np.float32)
    for i in range(NCORES):
        r = np.asarray(results[i]["out"])[0:T].astype(np.float32)  # [T, CPRIME]
        arr = r.reshape(T, 2, BPC, C).transpose(2, 1, 3, 0)  # [b, h, c, t]
        out[BPC * i : BPC * (i + 1)] = arr.reshape(BPC, C2, T)
    neg = np.finfo(np.float32).min
    for h, (_, _, empty) in enumerate(halves):
        if empty.any():
            out[:, h * C : (h + 1) * C, empty] = neg
    return out


def kernel(feature, segments):
    global LAST_RESULTS
    feature = np.ascontiguousarray(feature, dtype=np.float32)
    segments = np.ascontiguousarray(segments, dtype=np.float32)

    if "nc" not in _CACHE:
        _CACHE["nc"] = _build_module()
    nc = _CACHE["nc"]

    pk, halves = _host_pk(segments)
    fts = _shard_feature(feature)

    in_maps = []
    for i in range(NCORES):
        pk0 = pk[0].copy()
        pk0[:, 0:FT_BYTES] = fts[i].view(np.uint8)
        in_maps.append(
            {"pk0": pk0, "pk1": pk[1], "pk2": pk[2], "pk3": pk[3]}
        )

    res = run_bass_kernel_spmd(nc, in_maps, list(range(NCORES)), trace=TRACE)
    LAST_RESULTS = res
    return _unshard(res.results, halves)

